# revision 65
# baseline (speedup 1.0000x reference)
"""Distributed causal multi-head attention for TRN2 (8 NeuronCores).

Problem: B=2, T=2048, D=1024, H=16 heads (head_dim 64), causal MHA:
  q,k,v = x@W{q,k,v}+b, q *= dh**-0.5, o = softmax(mask(q k^T)) v, out = o@Wp + bp

Sharding: 8-way tensor parallel over heads for QKV+attention (core r
handles BOTH batches, heads {2r, 2r+1}), then an AllToAll re-shards by
TOKENS for the output projection: core r receives the full 1024 head
dims for its 64-token slice of every 512-token chunk and computes
out[tokens, ALL 1024 cols] with the full Wp.  A2A moves 8x less wire
data than the AllGather this replaces (measured AG: flat 17-38us per
op; the serial collective chain was the kernel's critical path), and
the projection becomes 8 N=1024 matmuls per chunk (batch pair packed
into the 128 stationary columns).  Per core:
  - QKV projections in fp16 on TensorE (q/k produced transposed [hd, t],
    v produced natural [t, hd] with an appended ones-column)
  - scores computed transposed [keys, q] (K=64 contraction, two heads
    packed into the 128x128 PE array via row tiling); ONE fused exp per
    key tile on ScalarE; causal handled by key-tile skipping + a
    post-exp 0/1 mask multiply on the diagonal blocks
  - AV uses exp-weights as the stationary operand -> o natural [q, hd]
    with per-partition row sums for free (ones column of v); normalize
    with a per-partition reciprocal; AV interleaves with scores at lag 1
  - o is PE-transposed locally so the bounce buffer carries oT in
    A2A shard-major order [token-group, hd, tokens]
  - chunks 0+1 share one AllToAll (ready before the warmup clears the
    serial collective chain); chunks 2/3 get their own.  Distinct
    bounce dram tensors per collective: tile tracks collective ins at
    whole-tensor granularity, and a shared tensor creates false WAR
    serialization.  Chunk projections run after attention, overlapping
    the in-flight late collectives (no PE slack exists mid-attention).
Host side shards/converts inputs, re-interleaves the token-sharded
outputs, and adds the output-constant bias terms (bv@Wp + bp; bk
cancels in softmax; bq is applied on device).
"""

import os
import numpy as np

B, T, D, H = 2, 2048, 1024, 16
DH = 64
NCORES = 8
HPC = H // NCORES      # heads per core = 2
CD = HPC * DH          # per-core head-dim = 128
P = 128
NCH = 4                # T chunks for the A2A pipeline
CHUNK = T // NCH       # 512
TG = CHUNK // NCORES   # per-core token group = 64
KT = T // P            # 16 key tiles
KD = D // P            # 8 contraction tiles for the projections

_CACHE = {}

# Results of the last device run (for test harnesses): BassKernelResults
LAST_RESULT = None


def _build_nc():
    import concourse.bass as bass
    import concourse.mybir as mybir
    import concourse.tile as tile
    from concourse import bacc
    from contextlib import ExitStack

    fp = mybir.dt.float16
    f32 = mybir.dt.float32
    AF = mybir.ActivationFunctionType

    nc = bacc.Bacc("TRN2", target_bir_lowering=False, debug=False,
                   num_devices=NCORES)

    xT = nc.dram_tensor("xT", [D, B, T], fp, kind="ExternalInput").ap()
    wq = nc.dram_tensor("wq", [D, CD], fp, kind="ExternalInput").ap()
    wk = nc.dram_tensor("wk", [D, CD], fp, kind="ExternalInput").ap()
    wv = nc.dram_tensor("wv", [D, CD], fp, kind="ExternalInput").ap()
    wp = nc.dram_tensor("wp", [D, D], fp, kind="ExternalInput").ap()
    bqp = nc.dram_tensor("bqp", [P, 1], f32, kind="ExternalInput").ap()
    maskf = nc.dram_tensor("maskf", [P, P], fp, kind="ExternalInput").ap()
    ident = nc.dram_tensor("ident", [P, P], fp, kind="ExternalInput").ap()
    # token-sharded output: core r owns batch r//4, rows
    # c*512 + (r%4)*128 .. +128 of each chunk c
    out = nc.dram_tensor("out", [NCH, P, D], f32,
                         kind="ExternalOutput").ap()

    # per-chunk A2A bounce/recv buffers (distinct tensors per chunk to
    # avoid false WAR serialization in tile's whole-tensor tracking).
    # Layout [shard j, batch, hd, tokens]: shard j carries MY two heads
    # for token group j; after A2A, slot j holds rank j's heads (d rows
    # j*128..) for MY token group -> exactly the projection's k-tiles.
    # A2A shard = (batch, 128-token subtile): core j receives batch
    # j//4, subtile j%4 of every chunk.  This lets the bounce carry o
    # NATURAL (plain partition-first DMA straight from the normalized
    # osb tile) - the oT transposes move to the projection tail where
    # the PE idles waiting for the late collectives.  Chunks 0+1 share
    # one A2A (ready long before the warmup clears the chain).
    TG2 = P  # tokens owned per core per chunk
    ob01 = nc.dram_tensor("ob01", [NCORES, 2, TG2, CD], fp).ap()
    ob2 = nc.dram_tensor("ob2", [NCORES, TG2, CD], fp).ap()
    ob3 = nc.dram_tensor("ob3", [NCORES, TG2, CD], fp).ap()
    # A2A outputs must be Local (shared outputs are AG/AR-only)
    g01 = nc.dram_tensor("g01", [NCORES, 2, TG2, CD], fp).ap()
    g2 = nc.dram_tensor("g2", [NCORES, TG2, CD], fp).ap()
    g3 = nc.dram_tensor("g3", [NCORES, TG2, CD], fp).ap()
    obc = [ob01[:, 0], ob01[:, 1], ob2, ob3]
    gathc = [g01[:, 0], g01[:, 1], g2, g3]
    cc_in = {1: ob01, 2: ob2, 3: ob3}
    cc_out = {1: g01, 2: g2, 3: g3}
    # warmup matches the data collectives' kind (AllToAll) so the
    # first-call staging cost is absorbed here, not by the first data op
    warm_in = nc.dram_tensor("warm_in", [NCORES, P], fp).ap()
    warm_out = nc.dram_tensor("warm_out", [NCORES, P], fp).ap()


    RG = [[0, 1, 2, 3, 4, 5, 6, 7]]

    with tile.TileContext(nc, num_cores=NCORES) as tc, ExitStack() as ctx:
        const = ctx.enter_context(tc.tile_pool(name="const", bufs=1))
        work = ctx.enter_context(tc.tile_pool(name="work", bufs=3))
        expp = ctx.enter_context(tc.tile_pool(name="expp", bufs=18))
        otkp = ctx.enter_context(tc.tile_pool(name="otkp", bufs=4))
        osbp = ctx.enter_context(tc.tile_pool(name="osbp", bufs=8))
        psum = ctx.enter_context(tc.tile_pool(name="psum", bufs=2,
                                              space="PSUM"))

        # ---- persistent SBUF ----
        xT_sb = const.tile([P, KD, B, T], fp)        # 64 KB/p
        wq_sb = const.tile([P, KD, CD], fp)
        wk_sb = const.tile([P, KD, CD], fp)
        wv_sb = const.tile([P, KD, CD], fp)
        wp_sb = const.tile([P, KD, D], fp)           # FULL Wp, 16 KB/p
        bq_sb = const.tile([P, 1], f32)
        mask_sb = const.tile([P, P], fp)             # 0/1 lower triangle
        ident_sb = const.tile([P, P], fp)
        qT_sb = const.tile([P, B, T], fp)            # 2 heads stacked
        kT_sb = const.tile([P, B, T], fp)
        v_sb = const.tile([P, KT, B, HPC, DH + 1], fp)

        xT_r = xT.rearrange("(k p) b t -> p k b t", p=P)
        # batch-0 chunk-0 data (k-split so the first matmuls start after
        # 512KB) + the three projection weights first, so the first
        # attention chunk starts as early as possible; the rest of x
        # streams behind; wp (not needed until the projection phase)
        # goes last
        nc.sync.dma_start(xT_sb[:, 0:4, 0, 0:512], xT_r[:, 0:4, 0, 0:512])
        nc.sync.dma_start(wq_sb[:], wq.rearrange("(k p) c -> p k c", p=P))
        nc.sync.dma_start(xT_sb[:, 4:8, 0, 0:512], xT_r[:, 4:8, 0, 0:512])
        nc.sync.dma_start(wk_sb[:], wk.rearrange("(k p) c -> p k c", p=P))
        nc.sync.dma_start(wv_sb[:], wv.rearrange("(k p) c -> p k c", p=P))
        nc.sync.dma_start(bq_sb[:], bqp)
        nc.sync.dma_start(mask_sb[:], maskf)
        nc.sync.dma_start(xT_sb[:, :, 1, 0:512], xT_r[:, :, 1, 0:512])
        nc.vector.memset(v_sb[:, :, :, :, DH:DH + 1], 1.0)
        for t4 in range(1, NCH):
            for b in range(B):
                nc.sync.dma_start(
                    xT_sb[:, :, b, t4 * 512:(t4 + 1) * 512],
                    xT_r[:, :, b, t4 * 512:(t4 + 1) * 512])
        nc.sync.dma_start(wp_sb[:], wp.rearrange("(k p) c -> p k c", p=P))
        # ident only feeds the projection-tail transposes: load last
        nc.sync.dma_start(ident_sb[:], ident)
        # tiny warmup collective: absorbs the first-collective latency
        # anomaly while the input DMAs stream
        nc.gpsimd.collective_compute(
            "AllToAll", bass.mybir.AluOpType.bypass,
            replica_groups=RG, ins=[warm_in], outs=[warm_out])

        def qkv_units(t4):
            """Projection work for T-chunk t4 as a list of closures, so
            it can be drip-fed into the attention k-loop (fills the PE
            while ScalarE paces the exp pipeline)."""
            units = []

            psqk_box = {}

            def q_unit(b):
                psqk = psum.tile([P, 1024], f32, tag="big", bufs=3,
                                 name=f"psqk_{t4}_{b}")
                psqk_box[b] = psqk
                for k in range(KD):
                    nc.tensor.matmul(
                        psqk[:, 0:512], wq_sb[:, k, :],
                        xT_sb[:, k, b, t4 * 512:(t4 + 1) * 512],
                        start=(k == 0), stop=(k == KD - 1))
                nc.vector.tensor_scalar_add(
                    qT_sb[:, b, t4 * 512:(t4 + 1) * 512], psqk[:, 0:512],
                    bq_sb[:, 0:1])

            def k_unit(b):
                psqk = psqk_box[b]
                for k in range(KD):
                    nc.tensor.matmul(
                        psqk[:, 512:1024], wk_sb[:, k, :],
                        xT_sb[:, k, b, t4 * 512:(t4 + 1) * 512],
                        start=(k == 0), stop=(k == KD - 1))
                nc.vector.tensor_copy(
                    kT_sb[:, b, t4 * 512:(t4 + 1) * 512], psqk[:, 512:1024])

            def v_unit(b, tt):
                psv = psum.tile([P, 256], f32, tag="big", bufs=3,
                                name=f"psv_{tt}_{b}")
                for k in range(KD):
                    nc.tensor.matmul(
                        psv[:, :CD], xT_sb[:, k, b, tt * P:(tt + 1) * P],
                        wv_sb[:, k, :], start=(k == 0),
                        stop=(k == KD - 1))
                nc.vector.tensor_copy(
                    out=v_sb[:, tt, b, :, 0:DH],
                    in_=psv[:, :CD].rearrange("p (h d) -> p h d", h=HPC))

            for b in range(B):
                units.append((t4, b, lambda b=b: q_unit(b)))
                units.append((t4, b, lambda b=b: k_unit(b)))
                for tt in range(4 * t4, 4 * t4 + 4):
                    units.append((t4, b, lambda b=b, tt=tt: v_unit(b, tt)))
            return units

        def attention_batch(c, b, filler=None):
            """Causal attention for q-chunk c, batch b (2 heads packed).

            scores for both heads go into one [128,1024] PSUM tile
            (row-packed K=64 matmuls -> halves), one fused exp per key
            tile, AV interleaved with lag 1.  AV accumulates all four
            q-subtiles of each head in one PSUM bank (4 interleaved
            accumulation groups as column ranges)."""
            nkt = 4 * (c + 1)
            exp_tiles = {}
            pso = {}
            for hh in range(2):
                pso[hh] = psum.tile([P, 4, DH + 1], f32, tag="o",
                                    name=f"pso_{c}_{b}_{hh}")

            def do_scores(k):
                ps_s = psum.tile([P, 1024], f32, tag="big", bufs=3,
                                 name=f"ps_{c}_{b}_{k}")
                j = k - 4 * c
                # diagonal tiles: queries < j*128 are entirely below
                # the causal boundary for this key tile - the AV skips
                # those s-blocks, so neither scores nor exp need them
                q0 = j * P if j >= 1 else 0
                for hh in range(2):
                    lo, hi = hh * DH, (hh + 1) * DH
                    nc.tensor.matmul(
                        ps_s[:, hh * 512 + q0:(hh + 1) * 512],
                        kT_sb[lo:hi, b, k * P:(k + 1) * P],
                        qT_sb[lo:hi, b, c * 512 + q0:(c + 1) * 512],
                        start=True, stop=True)
                e = expp.tile([P, 1024], fp, tag="expT",
                              name=f"expT_{c}_{b}_{k}")
                if j >= 1:
                    for hh in range(2):
                        lo = hh * 512 + q0
                        hi = (hh + 1) * 512
                        nc.scalar.activation(e[:, lo:hi], ps_s[:, lo:hi],
                                             AF.Exp)
                else:
                    nc.scalar.activation(e[:], ps_s[:], AF.Exp)
                if j >= 0:
                    blks = e[:].rearrange("p (hh q) -> p hh q", hh=2)[
                        :, :, j * P:(j + 1) * P]
                    nc.vector.tensor_mul(
                        blks, blks,
                        mask_sb[:, None, :].to_broadcast([P, 2, P]))
                exp_tiles[k] = e

            def do_av(k):
                # pso[hh] holds 4 interleaved accumulation groups in one
                # PSUM bank; only the first write of the bank (k==0,s==0)
                # may set start (bank-wide has_written clear).  For diag
                # key tiles (k>0) the mask-dependent s==j block goes last
                # so the other AV matmuls never queue behind the DVE mask.
                j = k - 4 * c
                order = list(range(4))
                if k > 0 and 0 <= j < 4:
                    order = [s for s in order if s != j] + [j]
                for hh in range(2):
                    for s in order:
                        if k <= 4 * c + s:
                            nc.tensor.matmul(
                                pso[hh][:, s, :],
                                exp_tiles[k][:, hh * 512 + s * P:
                                             hh * 512 + (s + 1) * P],
                                v_sb[:, k, b, hh, :],
                                start=(k == 0 and s == 0),
                                stop=(k == 4 * c + s),
                                skip_group_check=True)

            for k in range(nkt + 1):
                if k < nkt:
                    do_scores(k)
                if k > 0:
                    do_av(k - 1)
                if filler is not None and k >= 2 and (c == 0 or
                                                     k < nkt - 1):
                    # no fills near the end of the loop (except chunk 0,
                    # which feeds no collective): the A2A-critical finish
                    # work must not queue behind drip units
                    filler()
            return pso

        oTk = {}

        def proj_load(c, eng=None):
            """ONE DMA staging chunk c's A2A result (o natural,
            [my 128 tokens, rank k's 128 head dims])."""
            t_ = otkp.tile([P, KD, CD], fp, tag="oTnat",
                           name=f"oTnat_{c}")
            (eng or nc.sync).dma_start(
                t_[:], gathc[c].rearrange("k w cd -> w k cd"))
            oTk[c] = t_

        def proj_batch(c):
            """Transpose each received k-tile (the PE idles here
            waiting for the late collectives anyway), then project this
            core's 128-token subtile against the full Wp."""
            t_ = oTk[c]
            tT = work.tile([P, KD, CD], fp, tag="oTk", name=f"oTk_{c}")
            for k in range(KD):
                trp = psum.tile([P, P], fp, tag="o",
                                name=f"trp_{c}_{k}")
                nc.tensor.transpose(trp[:], t_[:, k, :], ident_sb[:])
                nc.vector.tensor_copy(tT[:, k, :], trp[:])
            psp = psum.tile([P, 1024], f32, tag="big", bufs=3,
                            name=f"psp_{c}")
            for k in range(KD):
                for hf in range(2):
                    # one matmul may write at most 512 f32 cols (one
                    # PSUM bank)
                    nc.tensor.matmul(
                        psp[:, hf * 512:(hf + 1) * 512], tT[:, k, :],
                        wp_sb[:, k, hf * 512:(hf + 1) * 512],
                        start=(k == 0), stop=(k == KD - 1))
            outsb = work.tile([P, 1024], f32, tag="outsb",
                              name=f"outsb_{c}")
            for hf in range(2):
                # per-half copy+write: the first out DMA overlaps the
                # second PSUM drain on the (serial) final-chunk tail
                nc.vector.tensor_copy(outsb[:, hf * 512:(hf + 1) * 512],
                                      psp[:, hf * 512:(hf + 1) * 512])
                nc.sync.dma_start(out[c][:, hf * 512:(hf + 1) * 512],
                                  outsb[:, hf * 512:(hf + 1) * 512])

        def finish_batch(c, b, pso):
            """normalize -> A2A bounce, o NATURAL (shard = (b, s))."""
            osb = osbp.tile([P, 4, CD], fp, tag="osb",
                            name=f"osb_{c}_{b}")
            for hh in range(2):
                for s in range(4):
                    rec = work.tile([P, 1], f32, tag="rec",
                                    name=f"rec_{c}_{b}_{hh}_{s}")
                    nc.vector.reciprocal(rec[:],
                                         pso[hh][:, s, DH:DH + 1])
                    nc.vector.tensor_scalar_mul(
                        osb[:, s, hh * DH:(hh + 1) * DH],
                        pso[hh][:, s, 0:DH], rec[:])
            # shard b*4+s carries osb[:, s, :] untransposed
            nc.sync.dma_start(
                obc[c][b * 4:(b + 1) * 4].rearrange("s w cd -> w s cd"),
                osb[:])

        # pipeline: attention(c) paces the compute; one AllToAll per
        # chunk fires the moment its bounce buffer is written; qkv(c+1)
        # is drip-fed INTO the attention k-loop.  Projections run after
        # attention, overlapping the in-flight late collectives (no PE
        # slack exists mid-attention).
        pending = []   # (chunk, batch, closure) qkv drip units

        def filler():
            if pending:
                pending.pop(0)[2]()
            if len(pending) > 8:
                pending.pop(0)[2]()

        def drain_for(c, b):
            while any(t == c and bb == b for t, bb, _ in pending):
                pending.pop(0)[2]()

        # only batch 0's projections block the first scores
        units0 = qkv_units(0)
        for _, _, u in units0[:6]:
            u()
        pending.extend(units0[6:])
        for c in range(NCH):
            pending.extend(qkv_units(c + 1) if c + 1 < NCH else [])
            for b in range(B):
                drain_for(c, b)
                pso = attention_batch(c, b, filler=filler)
                finish_batch(c, b, pso)
            if c in cc_in:
                nc.gpsimd.collective_compute(
                    "AllToAll", bass.mybir.AluOpType.bypass,
                    replica_groups=RG, ins=[cc_in[c]], outs=[cc_out[c]])
            if c == 2:
                # A2A 0/1 completed long ago: stage their results (one
                # DMA each) while chunk 3 computes
                proj_load(0)
                proj_load(1)
        while pending:
            pending.pop(0)[2]()
        # projection tail: chunks 0+1 fill the PE while the chunk 2/3
        # A2As land.  Their loads go on the gpsimd queue (idle after
        # the last trigger; a blocked load there can't delay the
        # bounce writes or out-writes on the sync queue).
        proj_load(2, eng=nc.gpsimd)
        proj_load(3, eng=nc.gpsimd)
        proj_batch(0)
        proj_batch(1)
        proj_batch(2)
        # keep the PE's HAM activity window busy while the final A2A
        # lands (~9us): an idle gap >3.4us re-throttles the PE to half
        # clock, doubling the cost of chunk 3's projection.  Scratch
        # matmuls on resident operands, no readers.
        scratch = psum.tile([P, 256], f32, tag="o", name="warmfill")
        for _ in range(80):
            nc.tensor.matmul(scratch[:], wq_sb[:, 0, :],
                             xT_sb[:, 0, 0, 0:256], start=True, stop=True)
        proj_batch(3)

    nc.finalize()
    return nc


def _get_nc():
    if "nc" not in _CACHE:
        _CACHE["nc"] = _build_nc()
    return _CACHE["nc"]


def kernel(x, Wq, bq, Wk, bk, Wv, bv, Wp, bp):
    global LAST_RESULT
    from concourse.bass_utils import run_bass_kernel_spmd

    x = np.asarray(x, dtype=np.float32)
    Wq = np.asarray(Wq, dtype=np.float32)
    Wk = np.asarray(Wk, dtype=np.float32)
    Wv = np.asarray(Wv, dtype=np.float32)
    Wp = np.asarray(Wp, dtype=np.float32)
    bq = np.asarray(bq, dtype=np.float32)
    bv = np.asarray(bv, dtype=np.float32)
    bp = np.asarray(bp, dtype=np.float32)

    s = DH ** -0.5
    maskf = np.where(
        np.arange(P)[:, None] <= np.arange(P)[None, :], 1.0, 0.0
    ).astype(np.float16)
    ident = np.eye(P, dtype=np.float16)
    xTg = np.ascontiguousarray(np.stack([x[0].T, x[1].T], axis=1)
                               ).astype(np.float16)
    wp16 = np.ascontiguousarray(Wp).astype(np.float16)

    in_maps = []
    for r in range(NCORES):
        cols = slice(r * CD, (r + 1) * CD)
        in_maps.append({
            "xT": xTg,
            "wq": (Wq[:, cols] * s).astype(np.float16),
            "wk": np.ascontiguousarray(Wk[:, cols]).astype(np.float16),
            "wv": np.ascontiguousarray(Wv[:, cols]).astype(np.float16),
            "wp": wp16,
            "bqp": np.ascontiguousarray((bq[cols] * s).reshape(P, 1)),
            "maskf": maskf,
            "ident": ident,
        })

    nc = _get_nc()
    res = run_bass_kernel_spmd(
        nc, in_maps, core_ids=list(range(NCORES)),
        trace=bool(int(os.environ.get("KERNEL_TRACE", "0"))))
    LAST_RESULT = res

    # token-sharded outputs: core r owns batch r//4, rows
    # c*512 + (r%4)*128 .. +128 of each chunk
    out = np.empty((B, T, D), dtype=np.float32)
    for r in range(NCORES):
        o = res.results[r]["out"]          # [NCH, 128, D]
        b, s = r // 4, r % 4
        for c in range(NCH):
            out[b, c * CHUNK + s * P:c * CHUNK + (s + 1) * P, :] = o[c]
    # bias terms that are constant w.r.t. the data path:
    #   v-bias passes through softmax rows (sum=1) -> + bv@Wp; plus bp.
    #   (bk shifts every logit in a row equally -> cancels in softmax.)
    out += (bv @ Wp + bp)[None, None, :]
    return out


# revision 66
# speedup vs baseline: 1.1695x; 1.1695x over previous
"""Distributed causal multi-head attention for TRN2 (8 NeuronCores).

Problem: B=2, T=2048, D=1024, H=16 heads (head_dim 64), causal MHA:
  q,k,v = x@W{q,k,v}+b, q *= dh**-0.5, o = softmax(mask(q k^T)) v, out = o@Wp + bp

Sharding: 8-way tensor parallel over heads for QKV+attention (core r
handles BOTH batches, heads {2r, 2r+1}), then an AllToAll re-shards by
TOKENS for the output projection: core r receives the full 1024 head
dims for its 64-token slice of every 512-token chunk and computes
out[tokens, ALL 1024 cols] with the full Wp.  A2A moves 8x less wire
data than the AllGather this replaces (measured AG: flat 17-38us per
op; the serial collective chain was the kernel's critical path), and
the projection becomes 8 N=1024 matmuls per chunk (batch pair packed
into the 128 stationary columns).  Per core:
  - QKV projections in fp16 on TensorE (q/k produced transposed [hd, t],
    v produced natural [t, hd] with an appended ones-column)
  - scores computed transposed [keys, q] (K=64 contraction, two heads
    packed into the 128x128 PE array via row tiling); ONE fused exp per
    key tile on ScalarE; causal handled by key-tile skipping + a
    post-exp 0/1 mask multiply on the diagonal blocks
  - AV uses exp-weights as the stationary operand -> o natural [q, hd]
    with per-partition row sums for free (ones column of v); normalize
    with a per-partition reciprocal; AV interleaves with scores at lag 1
  - o is PE-transposed locally so the bounce buffer carries oT in
    A2A shard-major order [token-group, hd, tokens]
  - chunks 0+1 share one AllToAll (ready before the warmup clears the
    serial collective chain); chunks 2/3 get their own.  Distinct
    bounce dram tensors per collective: tile tracks collective ins at
    whole-tensor granularity, and a shared tensor creates false WAR
    serialization.  Chunk projections run after attention, overlapping
    the in-flight late collectives (no PE slack exists mid-attention).
Host side shards/converts inputs, re-interleaves the token-sharded
outputs, and adds the output-constant bias terms (bv@Wp + bp; bk
cancels in softmax; bq is applied on device).
"""

import os
import numpy as np

B, T, D, H = 2, 2048, 1024, 16
DH = 64
NCORES = 8
HPC = H // NCORES      # heads per core = 2
CD = HPC * DH          # per-core head-dim = 128
P = 128
NCH = 4                # T chunks for the A2A pipeline
CHUNK = T // NCH       # 512
TG = CHUNK // NCORES   # per-core token group = 64
KT = T // P            # 16 key tiles
KD = D // P            # 8 contraction tiles for the projections

_CACHE = {}

# Results of the last device run (for test harnesses): BassKernelResults
LAST_RESULT = None


def _build_nc():
    import concourse.bass as bass
    import concourse.mybir as mybir
    import concourse.tile as tile
    from concourse import bacc
    from contextlib import ExitStack

    fp = mybir.dt.float16
    f32 = mybir.dt.float32
    AF = mybir.ActivationFunctionType

    nc = bacc.Bacc("TRN2", target_bir_lowering=False, debug=False,
                   num_devices=NCORES)

    xT = nc.dram_tensor("xT", [D, B, T], fp, kind="ExternalInput").ap()
    wq = nc.dram_tensor("wq", [D, CD], fp, kind="ExternalInput").ap()
    wk = nc.dram_tensor("wk", [D, CD], fp, kind="ExternalInput").ap()
    wv = nc.dram_tensor("wv", [D, CD], fp, kind="ExternalInput").ap()
    wp = nc.dram_tensor("wp", [D, D], fp, kind="ExternalInput").ap()
    bqp = nc.dram_tensor("bqp", [P, 1], f32, kind="ExternalInput").ap()
    maskf = nc.dram_tensor("maskf", [P, P], fp, kind="ExternalInput").ap()
    ident = nc.dram_tensor("ident", [P, P], fp, kind="ExternalInput").ap()
    # token-sharded output: core r owns batch r//4, rows
    # c*512 + (r%4)*128 .. +128 of each chunk c
    out = nc.dram_tensor("out", [NCH, P, D], f32,
                         kind="ExternalOutput").ap()

    # per-chunk A2A bounce/recv buffers (distinct tensors per chunk to
    # avoid false WAR serialization in tile's whole-tensor tracking).
    # Layout [shard j, batch, hd, tokens]: shard j carries MY two heads
    # for token group j; after A2A, slot j holds rank j's heads (d rows
    # j*128..) for MY token group -> exactly the projection's k-tiles.
    # A2A shard = (batch, 128-token subtile): core j receives batch
    # j//4, subtile j%4 of every chunk.  This lets the bounce carry o
    # NATURAL (plain partition-first DMA straight from the normalized
    # osb tile) - the oT transposes move to the projection tail where
    # the PE idles waiting for the late collectives.  Chunks 0+1 share
    # one A2A (ready long before the warmup clears the chain).
    TG2 = P  # tokens owned per core per chunk
    ob01 = nc.dram_tensor("ob01", [NCORES, 2, TG2, CD], fp).ap()
    ob2 = nc.dram_tensor("ob2", [NCORES, TG2, CD], fp).ap()
    ob3 = nc.dram_tensor("ob3", [NCORES, TG2, CD], fp).ap()
    # A2A outputs must be Local (shared outputs are AG/AR-only)
    g01 = nc.dram_tensor("g01", [NCORES, 2, TG2, CD], fp).ap()
    g2 = nc.dram_tensor("g2", [NCORES, TG2, CD], fp).ap()
    g3 = nc.dram_tensor("g3", [NCORES, TG2, CD], fp).ap()
    obc = [ob01[:, 0], ob01[:, 1], ob2, ob3]
    gathc = [g01[:, 0], g01[:, 1], g2, g3]
    cc_in = {1: ob01, 2: ob2, 3: ob3}
    cc_out = {1: g01, 2: g2, 3: g3}
    # warmup matches the data collectives' kind (AllToAll) so the
    # first-call staging cost is absorbed here, not by the first data op
    warm_in = nc.dram_tensor("warm_in", [NCORES, P], fp).ap()
    warm_out = nc.dram_tensor("warm_out", [NCORES, P], fp).ap()


    RG = [[0, 1, 2, 3, 4, 5, 6, 7]]

    with tile.TileContext(nc, num_cores=NCORES) as tc, ExitStack() as ctx:
        const = ctx.enter_context(tc.tile_pool(name="const", bufs=1))
        work = ctx.enter_context(tc.tile_pool(name="work", bufs=3))
        expp = ctx.enter_context(tc.tile_pool(name="expp", bufs=18))
        otkp = ctx.enter_context(tc.tile_pool(name="otkp", bufs=4))
        osbp = ctx.enter_context(tc.tile_pool(name="osbp", bufs=8))
        psum = ctx.enter_context(tc.tile_pool(name="psum", bufs=2,
                                              space="PSUM"))

        # ---- persistent SBUF ----
        xT_sb = const.tile([P, KD, B, T], fp)        # 64 KB/p
        wq_sb = const.tile([P, KD, CD], fp)
        wk_sb = const.tile([P, KD, CD], fp)
        wv_sb = const.tile([P, KD, CD], fp)
        wp_sb = const.tile([P, KD, D], fp)           # FULL Wp, 16 KB/p
        bq_sb = const.tile([P, 1], f32)
        mask_sb = const.tile([P, P], fp)             # 0/1 lower triangle
        ident_sb = const.tile([P, P], fp)
        qT_sb = const.tile([P, B, T], fp)            # 2 heads stacked
        kT_sb = const.tile([P, B, T], fp)
        v_sb = const.tile([P, KT, B, HPC, DH + 1], fp)

        xT_r = xT.rearrange("(k p) b t -> p k b t", p=P)
        # batch-0 chunk-0 data (k-split so the first matmuls start after
        # 512KB) + the three projection weights first, so the first
        # attention chunk starts as early as possible; the rest of x
        # streams behind; wp (not needed until the projection phase)
        # goes last
        nc.sync.dma_start(xT_sb[:, 0:4, 0, 0:512], xT_r[:, 0:4, 0, 0:512])
        nc.sync.dma_start(wq_sb[:], wq.rearrange("(k p) c -> p k c", p=P))
        nc.sync.dma_start(xT_sb[:, 4:8, 0, 0:512], xT_r[:, 4:8, 0, 0:512])
        nc.sync.dma_start(wk_sb[:], wk.rearrange("(k p) c -> p k c", p=P))
        nc.sync.dma_start(wv_sb[:], wv.rearrange("(k p) c -> p k c", p=P))
        nc.sync.dma_start(bq_sb[:], bqp)
        nc.sync.dma_start(mask_sb[:], maskf)
        nc.sync.dma_start(xT_sb[:, :, 1, 0:512], xT_r[:, :, 1, 0:512])
        nc.vector.memset(v_sb[:, :, :, :, DH:DH + 1], 1.0)
        for t4 in range(1, NCH):
            for b in range(B):
                nc.sync.dma_start(
                    xT_sb[:, :, b, t4 * 512:(t4 + 1) * 512],
                    xT_r[:, :, b, t4 * 512:(t4 + 1) * 512])
        nc.sync.dma_start(wp_sb[:], wp.rearrange("(k p) c -> p k c", p=P))
        # ident only feeds the projection-tail transposes: load last
        nc.sync.dma_start(ident_sb[:], ident)
        # tiny warmup collective: absorbs the first-collective latency
        # anomaly while the input DMAs stream
        nc.gpsimd.collective_compute(
            "AllToAll", bass.mybir.AluOpType.bypass,
            replica_groups=RG, ins=[warm_in], outs=[warm_out])

        def qkv_units(t4):
            """Projection work for T-chunk t4 as a list of closures, so
            it can be drip-fed into the attention k-loop (fills the PE
            while ScalarE paces the exp pipeline)."""
            units = []

            psqk_box = {}

            def q_unit(b):
                psqk = psum.tile([P, 1024], f32, tag="big", bufs=3,
                                 name=f"psqk_{t4}_{b}")
                psqk_box[b] = psqk
                for k in range(KD):
                    nc.tensor.matmul(
                        psqk[:, 0:512], wq_sb[:, k, :],
                        xT_sb[:, k, b, t4 * 512:(t4 + 1) * 512],
                        start=(k == 0), stop=(k == KD - 1))
                nc.vector.tensor_scalar_add(
                    qT_sb[:, b, t4 * 512:(t4 + 1) * 512], psqk[:, 0:512],
                    bq_sb[:, 0:1])

            def k_unit(b):
                psqk = psqk_box[b]
                for k in range(KD):
                    nc.tensor.matmul(
                        psqk[:, 512:1024], wk_sb[:, k, :],
                        xT_sb[:, k, b, t4 * 512:(t4 + 1) * 512],
                        start=(k == 0), stop=(k == KD - 1))
                nc.vector.tensor_copy(
                    kT_sb[:, b, t4 * 512:(t4 + 1) * 512], psqk[:, 512:1024])

            def v_unit(b, tt):
                psv = psum.tile([P, 256], f32, tag="big", bufs=3,
                                name=f"psv_{tt}_{b}")
                for k in range(KD):
                    nc.tensor.matmul(
                        psv[:, :CD], xT_sb[:, k, b, tt * P:(tt + 1) * P],
                        wv_sb[:, k, :], start=(k == 0),
                        stop=(k == KD - 1))
                nc.vector.tensor_copy(
                    out=v_sb[:, tt, b, :, 0:DH],
                    in_=psv[:, :CD].rearrange("p (h d) -> p h d", h=HPC))

            for b in range(B):
                units.append((t4, b, lambda b=b: q_unit(b)))
                units.append((t4, b, lambda b=b: k_unit(b)))
                for tt in range(4 * t4, 4 * t4 + 4):
                    units.append((t4, b, lambda b=b, tt=tt: v_unit(b, tt)))
            return units

        def attention_batch(c, b, filler=None):
            """Causal attention for q-chunk c, batch b (2 heads packed).

            scores for both heads go into one [128,1024] PSUM tile
            (row-packed K=64 matmuls -> halves), one fused exp per key
            tile, AV interleaved with lag 1.  AV accumulates all four
            q-subtiles of each head in one PSUM bank (4 interleaved
            accumulation groups as column ranges)."""
            nkt = 4 * (c + 1)
            exp_tiles = {}
            pso = {}
            for hh in range(2):
                pso[hh] = psum.tile([P, 4, DH + 1], f32, tag="o",
                                    name=f"pso_{c}_{b}_{hh}")

            def do_scores(k):
                ps_s = psum.tile([P, 1024], f32, tag="big", bufs=3,
                                 name=f"ps_{c}_{b}_{k}")
                j = k - 4 * c
                # diagonal tiles: queries < j*128 are entirely below
                # the causal boundary for this key tile - the AV skips
                # those s-blocks, so neither scores nor exp need them
                q0 = j * P if j >= 1 else 0
                for hh in range(2):
                    lo, hi = hh * DH, (hh + 1) * DH
                    nc.tensor.matmul(
                        ps_s[:, hh * 512 + q0:(hh + 1) * 512],
                        kT_sb[lo:hi, b, k * P:(k + 1) * P],
                        qT_sb[lo:hi, b, c * 512 + q0:(c + 1) * 512],
                        start=True, stop=True)
                e = expp.tile([P, 1024], fp, tag="expT",
                              name=f"expT_{c}_{b}_{k}")
                if j >= 1:
                    for hh in range(2):
                        lo = hh * 512 + q0
                        hi = (hh + 1) * 512
                        nc.scalar.activation(e[:, lo:hi], ps_s[:, lo:hi],
                                             AF.Exp)
                else:
                    nc.scalar.activation(e[:], ps_s[:], AF.Exp)
                if j >= 0:
                    blks = e[:].rearrange("p (hh q) -> p hh q", hh=2)[
                        :, :, j * P:(j + 1) * P]
                    nc.vector.tensor_mul(
                        blks, blks,
                        mask_sb[:, None, :].to_broadcast([P, 2, P]))
                exp_tiles[k] = e

            def do_av(k):
                # pso[hh] holds 4 interleaved accumulation groups in one
                # PSUM bank; only the first write of the bank (k==0,s==0)
                # may set start (bank-wide has_written clear).  For diag
                # key tiles (k>0) the mask-dependent s==j block goes last
                # so the other AV matmuls never queue behind the DVE mask.
                j = k - 4 * c
                order = list(range(4))
                if k > 0 and 0 <= j < 4:
                    order = [s for s in order if s != j] + [j]
                for hh in range(2):
                    for s in order:
                        if k <= 4 * c + s:
                            nc.tensor.matmul(
                                pso[hh][:, s, :],
                                exp_tiles[k][:, hh * 512 + s * P:
                                             hh * 512 + (s + 1) * P],
                                v_sb[:, k, b, hh, :],
                                start=(k == 0 and s == 0),
                                stop=(k == 4 * c + s),
                                skip_group_check=True)

            for k in range(nkt + 1):
                if k < nkt:
                    do_scores(k)
                if k > 0:
                    do_av(k - 1)
                if filler is not None and k >= 2 and (c == 0 or
                                                     k < nkt - 1):
                    # no fills near the end of the loop (except chunk 0,
                    # which feeds no collective): the A2A-critical finish
                    # work must not queue behind drip units
                    filler()
            return pso

        oTk = {}

        def proj_load(c, eng=None):
            """ONE DMA staging chunk c's A2A result (o natural,
            [my 128 tokens, rank k's 128 head dims])."""
            t_ = otkp.tile([P, KD, CD], fp, tag="oTnat",
                           name=f"oTnat_{c}")
            (eng or nc.sync).dma_start(
                t_[:], gathc[c].rearrange("k w cd -> w k cd"))
            oTk[c] = t_

        def proj_batch(c):
            """Transpose each received k-tile (the PE idles here
            waiting for the late collectives anyway), then project this
            core's 128-token subtile against the full Wp."""
            t_ = oTk[c]
            tT = work.tile([P, KD, CD], fp, tag="oTk", name=f"oTk_{c}")
            for k in range(KD):
                trp = psum.tile([P, P], fp, tag="o",
                                name=f"trp_{c}_{k}")
                nc.tensor.transpose(trp[:], t_[:, k, :], ident_sb[:])
                nc.vector.tensor_copy(tT[:, k, :], trp[:])
            psp = psum.tile([P, 1024], f32, tag="big", bufs=3,
                            name=f"psp_{c}")
            for k in range(KD):
                for hf in range(2):
                    # one matmul may write at most 512 f32 cols (one
                    # PSUM bank)
                    nc.tensor.matmul(
                        psp[:, hf * 512:(hf + 1) * 512], tT[:, k, :],
                        wp_sb[:, k, hf * 512:(hf + 1) * 512],
                        start=(k == 0), stop=(k == KD - 1))
            outsb = work.tile([P, 1024], f32, tag="outsb",
                              name=f"outsb_{c}")
            for hf in range(2):
                # per-half copy+write: the first out DMA overlaps the
                # second PSUM drain on the (serial) final-chunk tail
                nc.vector.tensor_copy(outsb[:, hf * 512:(hf + 1) * 512],
                                      psp[:, hf * 512:(hf + 1) * 512])
                nc.sync.dma_start(out[c][:, hf * 512:(hf + 1) * 512],
                                  outsb[:, hf * 512:(hf + 1) * 512])

        def finish_batch(c, b, pso):
            """normalize -> A2A bounce, o NATURAL (shard = (b, s))."""
            osb = osbp.tile([P, 4, CD], fp, tag="osb",
                            name=f"osb_{c}_{b}")
            for hh in range(2):
                for s in range(4):
                    rec = work.tile([P, 1], f32, tag="rec",
                                    name=f"rec_{c}_{b}_{hh}_{s}")
                    nc.vector.reciprocal(rec[:],
                                         pso[hh][:, s, DH:DH + 1])
                    nc.vector.tensor_scalar_mul(
                        osb[:, s, hh * DH:(hh + 1) * DH],
                        pso[hh][:, s, 0:DH], rec[:])
            # shard b*4+s carries osb[:, s, :] untransposed
            nc.sync.dma_start(
                obc[c][b * 4:(b + 1) * 4].rearrange("s w cd -> w s cd"),
                osb[:])

        # pipeline: attention(c) paces the compute; one AllToAll per
        # chunk fires the moment its bounce buffer is written; qkv(c+1)
        # is drip-fed INTO the attention k-loop.  Projections run after
        # attention, overlapping the in-flight late collectives (no PE
        # slack exists mid-attention).
        pending = []   # (chunk, batch, closure) qkv drip units

        def filler():
            if pending:
                pending.pop(0)[2]()
            if len(pending) > 8:
                pending.pop(0)[2]()

        def drain_for(c, b):
            while any(t == c and bb == b for t, bb, _ in pending):
                pending.pop(0)[2]()

        # only batch 0's projections block the first scores
        units0 = qkv_units(0)
        for _, _, u in units0[:6]:
            u()
        pending.extend(units0[6:])
        for c in range(NCH):
            pending.extend(qkv_units(c + 1) if c + 1 < NCH else [])
            for b in range(B):
                drain_for(c, b)
                pso = attention_batch(c, b, filler=filler)
                finish_batch(c, b, pso)
            if c in cc_in:
                nc.gpsimd.collective_compute(
                    "AllToAll", bass.mybir.AluOpType.bypass,
                    replica_groups=RG, ins=[cc_in[c]], outs=[cc_out[c]])
            if c == 2:
                # A2A 0/1 completed long ago: stage their results (one
                # DMA each) while chunk 3 computes
                proj_load(0)
                proj_load(1)
        while pending:
            pending.pop(0)[2]()
        # projection tail: chunks 0+1 fill the PE while the chunk 2/3
        # A2As land.  Their loads go on the gpsimd queue (idle after
        # the last trigger; a blocked load there can't delay the
        # bounce writes or out-writes on the sync queue).
        proj_load(2, eng=nc.gpsimd)
        proj_load(3, eng=nc.gpsimd)
        proj_batch(0)
        proj_batch(1)
        proj_batch(2)
        proj_batch(3)

    nc.finalize()
    return nc


def _get_nc():
    if "nc" not in _CACHE:
        _CACHE["nc"] = _build_nc()
    return _CACHE["nc"]


def kernel(x, Wq, bq, Wk, bk, Wv, bv, Wp, bp):
    global LAST_RESULT
    from concourse.bass_utils import run_bass_kernel_spmd

    x = np.asarray(x, dtype=np.float32)
    Wq = np.asarray(Wq, dtype=np.float32)
    Wk = np.asarray(Wk, dtype=np.float32)
    Wv = np.asarray(Wv, dtype=np.float32)
    Wp = np.asarray(Wp, dtype=np.float32)
    bq = np.asarray(bq, dtype=np.float32)
    bv = np.asarray(bv, dtype=np.float32)
    bp = np.asarray(bp, dtype=np.float32)

    s = DH ** -0.5
    maskf = np.where(
        np.arange(P)[:, None] <= np.arange(P)[None, :], 1.0, 0.0
    ).astype(np.float16)
    ident = np.eye(P, dtype=np.float16)
    xTg = np.ascontiguousarray(np.stack([x[0].T, x[1].T], axis=1)
                               ).astype(np.float16)
    wp16 = np.ascontiguousarray(Wp).astype(np.float16)

    in_maps = []
    for r in range(NCORES):
        cols = slice(r * CD, (r + 1) * CD)
        in_maps.append({
            "xT": xTg,
            "wq": (Wq[:, cols] * s).astype(np.float16),
            "wk": np.ascontiguousarray(Wk[:, cols]).astype(np.float16),
            "wv": np.ascontiguousarray(Wv[:, cols]).astype(np.float16),
            "wp": wp16,
            "bqp": np.ascontiguousarray((bq[cols] * s).reshape(P, 1)),
            "maskf": maskf,
            "ident": ident,
        })

    nc = _get_nc()
    res = run_bass_kernel_spmd(
        nc, in_maps, core_ids=list(range(NCORES)),
        trace=bool(int(os.environ.get("KERNEL_TRACE", "0"))))
    LAST_RESULT = res

    # token-sharded outputs: core r owns batch r//4, rows
    # c*512 + (r%4)*128 .. +128 of each chunk
    out = np.empty((B, T, D), dtype=np.float32)
    for r in range(NCORES):
        o = res.results[r]["out"]          # [NCH, 128, D]
        b, s = r // 4, r % 4
        for c in range(NCH):
            out[b, c * CHUNK + s * P:c * CHUNK + (s + 1) * P, :] = o[c]
    # bias terms that are constant w.r.t. the data path:
    #   v-bias passes through softmax rows (sum=1) -> + bv@Wp; plus bp.
    #   (bk shifts every logit in a row equally -> cancels in softmax.)
    out += (bv @ Wp + bp)[None, None, :]
    return out
